# revision 19
# baseline (speedup 1.0000x reference)
"""RNN-T decoder kernel for TRN2 (8 cores, T-sharded joint, replicated LSTM).

Layout notes
------------
B=8, T=128, U=64, E=512, H=1024 (8 k-chunks), J=640 (5 j-chunks), OD=1024.
Each core handles T-slice [16c, 16c+16) of the joint; the 2-layer LSTM over U
is computed identically (replicated, all 8 batches) on every core.

Gate permutation: hidden dim is split in 4 quarters (col-tile groups). Group
j's 1024 gate columns are [i_j | f_j | o_j | g_j] (256 each), where x_j acts
on hidden units [256j, 256j+256). Weights/bias/X tensors are host-permuted
to this order.

Gates PSUM tile (128, 1024): group j occupies partitions [32j, 32j+8)
(batch-major), accumulated by 4-way column-packed matmuls (tile_position).
Column half hf=0 holds [i|f], hf=1 holds [o|g]; the hf=0 accumulation group
finishes first so sigmoid(i,f) overlaps the hf=1 matmul stream.

The joint is interleaved with the LSTM: as soon as layer-1 finishes a
16-step u-block, zd and the 16 output blocks (8 (b,t)-pairs x 16 u rows
each) for that u-block are emitted, keeping the PE dense/warm and the
output DMA spread over the whole kernel. Output is fp16 (host upcasts).
"""
import numpy as np
import ml_dtypes

import concourse.bass as bass
import concourse.bacc as bacc
import concourse.mybir as mybir
import concourse.tile as tile

dt = mybir.dt
AF = mybir.ActivationFunctionType

B, T, E, H, J, OD = 8, 128, 512, 1024, 640, 1024
HK = H // 128   # 8 h-chunks
JC = J // 128   # 5 j-chunks
EK = E // 128   # 4 e-chunks
TSH = T // 8    # 16 t per core
NG = 4          # col-tile groups


def bcast_mid(ap, count):
    """(128, N) AP -> (128, count, N) with a 0-step middle dim."""
    return bass.AP(ap.tensor, ap.offset, [ap.ap[0], [0, count], ap.ap[1]])


def build_program(U=64, n_cores=8, with_biases=False,
                  with_out_bias=False):
    nc = bacc.Bacc("TRN2", target_bir_lowering=False, debug=False,
                   num_devices=n_cores)
    f16, f32, i32 = dt.float16, dt.float32, dt.int32
    UG = U // 16  # u-blocks of 16
    assert U % 16 == 0

    # ---------------- external inputs ----------------
    x0in_d = nc.dram_tensor("x0in", [U, B, 4096], f16, kind="ExternalInput")
    zein_d = nc.dram_tensor("zein", [128, JC, B * TSH], f16, kind="ExternalInput")
    wih1_d = nc.dram_tensor("wih1t", [16, 128, HK, 256], f16, kind="ExternalInput")
    whh0_d = nc.dram_tensor("whh0t", [128, HK, NG, 1024], f16, kind="ExternalInput")
    whh1_d = nc.dram_tensor("whh1t", [128, HK, NG, 1024], f16, kind="ExternalInput")
    inj_d = nc.dram_tensor("inj8", [8, 8], f16, kind="ExternalInput")
    eye128_d = nc.dram_tensor("eye128", [128, 128], f16, kind="ExternalInput")
    wdec_d = nc.dram_tensor("wdect", [128, HK, JC, 128], f16, kind="ExternalInput")
    wout_d = nc.dram_tensor("woutt", [128, JC, OD], f16, kind="ExternalInput")
    bout_d = nc.dram_tensor("boutrep", [128, OD], f32, kind="ExternalInput")
    # layer-1 (b_ih + b_hh), gate-permuted, replicated over partitions
    bi1_d = nc.dram_tensor("bihh1", [128, 4096], f16, kind="ExternalInput")

    out_d = nc.dram_tensor("out", [B * TSH, U, OD], f16, kind="ExternalOutput")

    # ---------------- internal dram ----------------
    x1_d = nc.dram_tensor("X1d", [U, B, 4096], f16)

    with tile.TileContext(nc) as tc:
        with tc.tile_pool(name="const", bufs=1) as pc:
            # constants
            inj_sb = pc.tile([8, 8], f16, tag="inj")
            nc.sync.dma_start(inj_sb[:], inj_d.ap())
            eye128_sb = pc.tile([128, 128], f16, tag="eye128")
            nc.sync.dma_start(eye128_sb[:], eye128_d.ap())
            if with_biases:
                bi1_sb = pc.tile([128, 4096], f16, tag="bi1")
                nc.sync.dma_start(bi1_sb[:], bi1_d.ap())
            # joint weights, resident for the whole kernel (loaded after
            # the whh chunks below -- not needed until the first zd/jblk)
            wdec_sb = pc.tile([128, HK, JC, 128], f16, tag="wdec")
            wout_sb = pc.tile([128, JC, OD], f16, tag="wout")
            if with_out_bias:
                bout_sb = pc.tile([128, OD], f32, tag="bouts")
                nc.sync.dma_start(bout_sb[:], bout_d.ap())
            ze_sb = pc.tile([128, JC, B * TSH], f16, tag="ze")
            zd_sb = pc.tile([128, JC, U, B], f16, tag="zd")
            # h_dec transposed history ring (32 steps), both layers (fp16)
            RING = 32
            hdec = [pc.tile([128, HK, RING, B], f16, tag=f"hdec{l}",
                            name=f"hdec{l}") for l in range(2)]

            # ---------------- P1: weight/zе loads ----------------
            pw_ctx = tc.tile_pool(name="whh", bufs=1)
            pw = pw_ctx.__enter__()
            # recurrent weights (resident for whole LSTM)
            whh_sb = [pw.tile([128, HK, NG, 1024], f16, tag=f"whh{l}",
                               name=f"whh{l}") for l in range(2)]
            for l, wd in ((0, whh0_d), (1, whh1_d)):
                for kc in range(HK):
                    eng = nc.sync if kc % 2 == 0 else nc.scalar
                    eng.dma_start(whh_sb[l][:, kc:kc + 1],
                                  wd.ap()[:, kc:kc + 1])
            nc.sync.dma_start(wdec_sb[:], wdec_d.ap())
            nc.scalar.dma_start(wout_sb[:], wout_d.ap())
            nc.scalar.dma_start(ze_sb[:], zein_d.ap())

            # ---------------- P3: LSTM + interleaved joint ----------------
            with (
                tc.tile_pool(name="lstmS", bufs=1) as lS,
                tc.tile_pool(name="lstmPS", bufs=1, space="PSUM") as lP,
            ):
                gate_ps = [lP.tile([128, 1024], f32, tag=f"gates{l}",
                                   name=f"gates{l}") for l in range(2)]
                nc.vector.memset(gate_ps[0][:], 0.0)
                nc.vector.memset(gate_ps[1][:], 0.0)
                czero = [lS.tile([128, 256], f16, tag=f"c{l}", name=f"cz{l}",
                                  bufs=2) for l in range(2)]
                nc.gpsimd.memset(czero[0][:], 0.0)
                nc.gpsimd.memset(czero[1][:], 0.0)
                cprev = [czero[0], czero[1]]
                xsrc = [x0in_d, x1_d]
                xf_t = {}

                def lstm_mm_hf(l, u, hf):
                    pg = gate_ps[l]
                    if hf == 0:
                        # X fetch (prefetchable; no recurrent dep)
                        xf = lS.tile([8, 4096], f16, tag="xf", bufs=2)
                        nc.gpsimd.dma_start(xf[:], xsrc[l].ap()[u])
                        xf_t[l] = xf
                    xf = xf_t[l]
                    sl = slice(hf * 512, (hf + 1) * 512)
                    # inject OPENS the accumulation group: it depends only on
                    # the prefetched X, so it runs in the PE bubble while the
                    # previous step's activation chain produces h
                    for j in range(NG):
                        nc.tensor.matmul(
                            pg[32 * j:32 * j + 8, sl], inj_sb[:],
                            xf[:, j * 1024 + hf * 512:
                               j * 1024 + (hf + 1) * 512],
                            tile_position=(0, 32 * j),
                            start=True, stop=(u == 0))
                    if u > 0:
                        for kc in range(HK):
                            for j in range(NG):
                                nc.tensor.matmul(
                                    pg[32 * j:32 * j + 8, sl],
                                    hdec[l][:, kc, (u - 1) % RING, :],
                                    whh_sb[l][:, kc, j,
                                              hf * 512:(hf + 1) * 512],
                                    tile_position=(0, 32 * j),
                                    start=False, stop=(kc == HK - 1))

                def act_sif(l):
                    # sigmoid(i|f) — cols [0:512) of the gates psum
                    pg = gate_ps[l]
                    sif = lS.tile([128, 512], f16, tag=f"sif{l}")
                    nc.scalar.activation(sif[:], pg[:, 0:512], AF.Sigmoid)
                    return sif

                def mul_fc(l, sif):
                    # t2 = sigmoid(f) * c_prev   (overlaps hf=1 matmuls)
                    t2 = lS.tile([128, 256], f16, tag=f"t2{l}")
                    nc.vector.tensor_mul(t2[:], sif[:, 256:512], cprev[l][:])
                    return t2

                def act_og(l):
                    pg = gate_ps[l]
                    tg = lS.tile([128, 256], f16, tag=f"tg{l}")
                    nc.scalar.activation(tg[:], pg[:, 768:1024], AF.Tanh)
                    so = lS.tile([128, 256], f16, tag=f"so{l}")
                    nc.scalar.activation(so[:], pg[:, 512:768], AF.Sigmoid)
                    return tg, so

                def cell_update(l, sif, t2, tg):
                    t1 = lS.tile([128, 256], f16, tag=f"t1{l}")
                    nc.vector.tensor_mul(t1[:], sif[:, 0:256], tg[:])
                    cnew = lS.tile([128, 256], f16, tag=f"c{l}", bufs=2)
                    nc.vector.tensor_add(cnew[:], cnew_in(t2), t1[:])
                    cprev[l] = cnew
                    return cnew

                def cnew_in(t2):
                    return t2[:]

                def lstm_tail(l, u, so, cnew):
                    tc_ = lS.tile([128, 256], f16, tag=f"tc{l}")
                    nc.scalar.activation(tc_[:], cnew[:], AF.Tanh)
                    h = lS.tile([128, 256], f16, tag=f"h{l}", bufs=2)
                    nc.vector.tensor_mul(h[:], so[:], tc_[:])
                    ht = lP.tile([128, 256], f16, tag="ht", bufs=2)
                    for cb in range(2):
                        nc.tensor.transpose(
                            ht[:, cb * 128:(cb + 1) * 128],
                            h[:, cb * 128:(cb + 1) * 128],
                            eye128_sb[:])
                    # one fused copy: src (p, cb, j, b) -> hdec[p, 2j+cb, u, b]
                    hd = hdec[l][:, 0, u % RING, :]  # (128, B) at kc=0
                    dst = bass.AP(hd.tensor, hd.offset,
                                  [hd.ap[0], [RING * B, 2],
                                   [2 * RING * B, NG], [1, B]])
                    src_ap = bass.AP(ht[:].tensor, ht[:].offset,
                                     [ht[:].ap[0], [128, 2], [32, NG], [1, B]])
                    nc.vector.tensor_copy(dst, src_ap)

                def x1_chunk(kb, nc2):
                    hd0 = hdec[0]
                    w1c = lS.tile([128, HK, 256], f16, tag="w1c", bufs=2)
                    eng = nc.scalar if nc2 % 2 == 0 else nc.sync
                    eng.dma_start(w1c[:], wih1_d.ap()[nc2])
                    ps = lP.tile([128, 512], f32, tag="jp", bufs=2)
                    psv = ps[:, 0:256]
                    for kc in range(HK):
                        nc.tensor.matmul(
                            psv,
                            hd0[:, kc, (kb * 16) % RING:
                                (kb * 16) % RING + 16, :],
                            w1c[:, kc, :],
                            start=(kc == 0), stop=(kc == HK - 1))
                    x1c = lS.tile([128, 256], f16, tag="x1c", bufs=2)
                    if with_biases:
                        nc.vector.tensor_add(
                            x1c[:], psv,
                            bi1_sb[:, nc2 * 256:(nc2 + 1) * 256])
                    else:
                        nc.vector.tensor_copy(x1c[:], psv)
                    nc.gpsimd.dma_start(
                        x1_d.ap()[kb * 16:(kb + 1) * 16, :,
                                  nc2 * 256:(nc2 + 1) * 256],
                        x1c[:])

                def zd_chunk(kb, jc):
                    u0 = kb * 16
                    zp = lP.tile([128, 512], f32, tag="jp", bufs=2)
                    zpv = zp[:, 0:128]
                    for kc in range(HK):
                        nc.tensor.matmul(
                            zpv, wdec_sb[:, kc, jc, :],
                            hdec[1][:, kc, u0 % RING:u0 % RING + 16, :]
                            .rearrange("p u b -> p (u b)"),
                            start=(kc == 0), stop=(kc == HK - 1))
                    nc.vector.tensor_copy(
                        zd_sb[:, jc, u0:u0 + 16, :]
                        .rearrange("p u b -> p (u b)"), zpv)

                def jblk(i):
                    # output block: 8 (b,tl)-pairs x 16 u rows
                    kb, sub = i // 16, i % 16
                    u0 = kb * 16
                    pr0 = sub * 8
                    b = sub // 2
                    zjt = lS.tile([128, JC, 128], f16, tag="zjt", bufs=2)
                    zj = lS.tile([128, JC, 128], f16, tag="zj", bufs=2)
                    # fused add over all 5 j-chunks: (p, jc, pair, u)
                    zea = ze_sb[:, 0, pr0:pr0 + 8]
                    ze_bc = bass.AP(zea.tensor, zea.offset,
                                    [zea.ap[0], [B * TSH, JC], [1, 8],
                                     [0, 16]])
                    zda = zd_sb[:, 0, u0:u0 + 16, b]
                    zd_bc = bass.AP(zda.tensor, zda.offset,
                                    [zda.ap[0], [U * B, JC], [0, 8],
                                     [B, 16]])
                    nc.vector.tensor_tensor(
                        zjt[:].rearrange("p jc (a u) -> p jc a u", a=8),
                        ze_bc, zd_bc, op=mybir.AluOpType.add)
                    nc.scalar.activation(
                        zj[:].rearrange("p jc n -> p (jc n)"),
                        zjt[:].rearrange("p jc n -> p (jc n)"), AF.Tanh)
                    osb = lS.tile([128, OD], f16, tag="osb", bufs=2)
                    opsl = [lP.tile([128, 512], f32, tag="jp", bufs=2,
                                    name=f"ops{n2}") for n2 in range(2)]
                    for jc in range(JC):
                        for n2 in range(2):
                            nc.tensor.matmul(
                                opsl[n2][:],
                                zj[:, jc, :],
                                wout_sb[:, jc, n2 * 512:(n2 + 1) * 512],
                                start=(jc == 0), stop=(jc == JC - 1))
                    for n2 in range(2):
                        ops_ = opsl[n2]
                        if with_out_bias:
                            nc.vector.tensor_add(
                                osb[:, n2 * 512:(n2 + 1) * 512], ops_[:],
                                bout_sb[:, n2 * 512:(n2 + 1) * 512])
                        elif n2 == 0:
                            nc.vector.tensor_copy(
                                osb[:, n2 * 512:(n2 + 1) * 512], ops_[:])
                        else:
                            nc.scalar.activation(
                                osb[:, n2 * 512:(n2 + 1) * 512], ops_[:],
                                AF.Identity)
                    nc.sync.dma_start(
                        out_d.ap()[pr0:pr0 + 8, u0:u0 + 16, :], osb[:])

                LAG = 24
                emitted = 0
                for u in range(U + LAG):
                    steps = []
                    if u < U:
                        steps.append((0, u))
                    if u >= LAG:
                        steps.append((1, u - LAG))
                    with nc.named_scope(f"w{u:02d}"):
                        for l, uu in steps:
                            lstm_mm_hf(l, uu, 0)
                        sif = {l: act_sif(l) for l, uu in steps}
                        for l, uu in steps:
                            lstm_mm_hf(l, uu, 1)
                        t2 = {l: mul_fc(l, sif[l]) for l, uu in steps}
                        togs = {l: act_og(l) for l, uu in steps}
                        for l, uu in steps:
                            cn = cell_update(l, sif[l], t2[l], togs[l][0])
                            lstm_tail(l, uu, togs[l][1], cn)
                    zdrdy = LAG + 15
                    # x1: 2 chunks per iteration over 8 iters per block
                    for kb in range(UG):
                        s0 = 16 * kb + 15
                        if s0 <= u <= s0 + 7:
                            with nc.named_scope(f"x1b{kb}"):
                                x1_chunk(kb, 2 * (u - s0))
                                x1_chunk(kb, 2 * (u - s0) + 1)
                    # zd: 1 chunk per iteration over 5 iters per block
                    for kb in range(UG):
                        z0 = zdrdy + 16 * kb
                        if z0 <= u <= z0 + 4:
                            with nc.named_scope(f"zd{kb}"):
                                zd_chunk(kb, u - z0)
                    avail = 16 * sum(
                        1 for kb in range(UG) if u >= zdrdy + 16 * kb + 4)
                    in_x1 = 47 <= u <= 54
                    target = min(avail, emitted + (1 if in_x1 else 2))
                    while emitted < target:
                        with nc.named_scope(f"jb{emitted:02d}"):
                            jblk(emitted)
                        emitted += 1
                # drain: remaining zd chunks of the last block, then jblks
                z0 = zdrdy + 16 * (UG - 1)
                for jc in range(JC):
                    if z0 + jc > U + LAG - 1:
                        with nc.named_scope(f"zd{UG - 1}"):
                            zd_chunk(UG - 1, jc)
                while emitted < 16 * UG:
                    with nc.named_scope(f"jb{emitted:02d}"):
                        jblk(emitted)
                    emitted += 1

            pw_ctx.__exit__(None, None, None)

    nc.compile()
    return nc


# ---------------- host-side prep ----------------

def gate_perm():
    """perm[j*1024 + s] -> row index in torch (i,f,g,o) 4H gate layout,
    with group-local order [i|f|o|g]."""
    perm = np.zeros(4 * H, dtype=np.int64)
    for j in range(NG):
        base = j * 1024
        hid = np.arange(256) + j * 256
        perm[base + 0:base + 256] = 0 * H + hid      # i
        perm[base + 256:base + 512] = 1 * H + hid    # f
        perm[base + 512:base + 768] = 3 * H + hid    # o
        perm[base + 768:base + 1024] = 2 * H + hid   # g
    return perm


def prep_inputs(hs_pad, ys_in_pad, embed, W_ih0, W_hh0, b_ih0, b_hh0,
                W_ih1, W_hh1, b_ih1, b_hh1, W_enc, b_enc, W_dec, W_out, b_out,
                U=64, n_cores=8):
    perm = gate_perm()

    def wiht(W, KD, KC):  # (4H, KD) -> (128, KC, 4096) fp16, permuted gates
        Wp = W[perm]                      # (4096, KD)
        return np.ascontiguousarray(
            Wp.T.reshape(KC, 128, 4096).transpose(1, 0, 2)).astype(np.float16)

    def whht(W):  # (4H, H) -> (128, HK, NG, 1024) fp16
        Wp = W[perm]                      # (4096, 1024) rows=permuted gates
        # [p, kc, j, n] = Wp[j*1024+n, kc*128+p]
        a = Wp.T.reshape(HK, 128, NG, 1024).transpose(1, 0, 2, 3)
        return np.ascontiguousarray(a).astype(np.float16)

    ins = {}
    ys = np.asarray(ys_in_pad).astype(np.int64)   # (B, U)
    # host: X0 = embed[ys] @ W_ih0[perm].T + bias0  -> (U, B, 4096)
    eys = np.asarray(embed, np.float32)[ys]           # (B, U, E)
    Wp0 = np.asarray(W_ih0, np.float32)[perm]         # (4096, E)
    bias0 = (np.asarray(b_ih0, np.float32)
             + np.asarray(b_hh0, np.float32))[perm]
    x0 = np.einsum("bue,ge->ubg", eys, Wp0) + bias0
    ins["x0in"] = np.ascontiguousarray(x0).astype(np.float16)
    w1 = wiht(W_ih1, H, HK)  # (128, HK, 4096)
    ins["wih1t"] = np.ascontiguousarray(
        w1.reshape(128, HK, 16, 256).transpose(2, 0, 1, 3))
    ins["whh0t"] = whht(W_hh0)
    ins["whh1t"] = whht(W_hh1)
    ins["inj8"] = np.eye(8, dtype=np.float16)
    ins["eye128"] = np.eye(128, dtype=np.float16)
    # [p, ec, jc, m] = W[jc*128+m, ec*128+p]
    def wjt(W, KC):
        a = W.T.reshape(KC, 128, JC, 128).transpose(1, 0, 2, 3)
        return np.ascontiguousarray(a).astype(np.float16)
    ins["wdect"] = wjt(W_dec, HK)
    # [p, jc, od] = W_out[od, jc*128+p]
    ins["woutt"] = np.ascontiguousarray(
        W_out.T.reshape(JC, 128, OD).transpose(1, 0, 2)).astype(np.float16)
    ins["boutrep"] = np.tile(np.asarray(b_out, np.float32)[None, :], (128, 1))
    ins["bihh1"] = np.tile(((b_ih1 + b_hh1)[perm]).astype(np.float16)[None, :],
                           (128, 1))

    # host: ze = hs @ W_enc.T + b_enc  -> per-core (128, JC, B*TSH)
    ze_full = (np.asarray(hs_pad, np.float32)
               @ np.asarray(W_enc, np.float32).T
               + np.asarray(b_enc, np.float32))     # (B, T, J)
    maps = []
    for c in range(n_cores):
        m = dict(ins)
        zs = ze_full[:, TSH * c:TSH * (c + 1), :]   # (B, TSH, J)
        m["zein"] = np.ascontiguousarray(
            zs.reshape(B * TSH, JC, 128).transpose(2, 1, 0)
        ).astype(np.float16)
        maps.append(m)
    return maps


def gather_output(results):
    outs = [np.asarray(r["out"], dtype=np.float32).reshape(B, TSH, -1, OD)
            for r in results]
    return np.concatenate(outs, axis=1)


# ---------------- entry point ----------------
import sys as _sys
import types as _types

# Recreate the missing antenv.axon_hooks so trace=True works under axon
# (used only when BASS_TRACE=1 is set by a profiling harness).
if "antenv.axon_hooks" not in _sys.modules:
    _m = _types.ModuleType("antenv.axon_hooks")

    def _get_hook():
        try:
            from trn_agent_boot.trn_boot import _ntff_profile_via_ctypes
            return _ntff_profile_via_ctypes("/opt/axon/libaxon_pjrt.so")
        except Exception:
            return None
    _m.get_axon_ntff_profile_hook = _get_hook
    _sys.modules["antenv.axon_hooks"] = _m

_NC = None
last_results = None


def kernel(**inputs):
    """Full-input RNN-T decoder: returns (B, T, U, ODIM) float32."""
    global _NC, last_results
    from concourse.bass_utils import run_bass_kernel_spmd
    U = int(np.asarray(inputs["ys_in_pad"]).shape[1])
    wb = any(float(np.abs(np.asarray(inputs[k])).max()) != 0.0
             for k in ("b_ih0", "b_hh0", "b_ih1", "b_hh1"))
    wob = float(np.abs(np.asarray(inputs["b_out"])).max()) != 0.0
    if _NC is None:
        _NC = build_program(U=U, n_cores=8, with_biases=wb, with_out_bias=wob)
    maps = prep_inputs(**inputs, U=U)
    res = run_bass_kernel_spmd(_NC, maps, core_ids=list(range(8)))
    last_results = res
    return gather_output(res.results)
